# revision 2
# baseline (speedup 1.0000x reference)
"""Cross-attention kernel for Trainium2, sharded over 8 NeuronCores.

Problem (hardcoded): b=4, n=m=2048, query_dim=context_dim=512,
heads=8, dim_head=64 (inner=512), f32 I/O.

Sharding: data-parallel over (batch, query-half): core c -> batch c//2,
query rows [(c%2)*1024, (c%2+1)*1024). Each core holds the full K/V
context for its batch, so there are no collectives and output shards
tile the full output exactly.

v2 design (ACT-bound pipeline):
  - The softmax exp is the hard floor: 8 heads * 1024n * 2048m = 16.7M
    elements through ScalarE at 1 elem/lane/cycle @1.2GHz ~= 133us.
    Everything else is organized to hide under a continuous ACT stream.
  - Scores per head-pair via two row-tiled (K=64) matmuls running
    CONCURRENTLY in the PE array (tile_position (0,0)/(64,0)): head 2ic
    lives in partitions 0:64 of qT/kT, head 2ic+1 in 64:128. Halves
    score PE time vs the zero-padded full-K form.
  - nj-outer loop (two 512-query passes); per (nj, pair, mi) one exp
    instruction covers both heads [128, 1024].
  - Software pipelining: scores run 2 m-chunks ahead of attn@V so the
    PE never waits on exp; Q/K/V projections are emitted as fillers
    inside the first pair's attention stream.
  - attn@V keeps the ones-column trick: v tiles [128, h, 128] =
    [V_h | 1 | 0pad]; psum row 64 = softmax denominator.
  - PSUM budget (8 banks): scores 2x[128,1024] (4) + o [128,1024] (2)
    + proj/outproj [128,512] x2 (2). o is freed fast via a DVE copy to
    SBUF; normalization (recip + DMA broadcast + mul) runs from SBUF.
"""

import numpy as np
import ml_dtypes

import concourse.bass as bass
import concourse.mybir as mybir
import concourse.tile as tile
from concourse import bacc
from concourse.bass_utils import run_bass_kernel_spmd

BF16 = mybir.dt.bfloat16
F32 = mybir.dt.float32

B, N, M = 4, 2048, 2048
CDIM, INNER = 512, 512
H, D = 8, 64
NSH = N // 2  # query rows per core
N_CORES = 8
SCALE = D ** -0.5

CC = CDIM // 128   # contraction chunks for projections (4)
IC = INNER // 128  # inner-dim chunks (4)
MT = M // 128      # m tiles (16)
NJ = NSH // 512    # n chunks of 512 (2)
NT = NSH // 128    # n tiles (8)
MJ = M // 512      # m chunks of 512 (4)


def build_nc() -> bass.Bass:
    nc = bacc.Bacc(None)

    pixelT = nc.dram_tensor("pixelT", [CDIM, NSH], BF16, kind="ExternalInput")
    patchT = nc.dram_tensor("patchT", [CDIM, M], BF16, kind="ExternalInput")
    wq = nc.dram_tensor("wq", [CDIM, INNER], BF16, kind="ExternalInput")
    wk = nc.dram_tensor("wk", [CDIM, INNER], BF16, kind="ExternalInput")
    wv = nc.dram_tensor("wv", [CDIM, INNER], BF16, kind="ExternalInput")
    wo = nc.dram_tensor("wo", [INNER, CDIM], BF16, kind="ExternalInput")
    bo = nc.dram_tensor("bo", [CDIM], F32, kind="ExternalInput")
    out = nc.dram_tensor("out", [NSH, CDIM], F32, kind="ExternalOutput")

    with tile.TileContext(nc) as tc:
        with (
            tc.tile_pool(name="weights", bufs=1) as wpool,
            tc.tile_pool(name="acts", bufs=1) as apool,
            tc.tile_pool(name="qkv", bufs=1) as qkvpool,
            tc.tile_pool(name="vsb", bufs=MT) as vpool,
            tc.tile_pool(name="attn", bufs=4) as atpool,
            tc.tile_pool(name="norm", bufs=2) as npool,
            tc.tile_pool(name="small", bufs=2) as rpool,
            tc.tile_pool(name="stage", bufs=2) as stpool,
        ):
            # ---- load weights + activations -------------------------------
            wq_sb = wpool.tile([128, CC, INNER], BF16, tag="wq")
            wk_sb = wpool.tile([128, CC, INNER], BF16, tag="wk")
            wv_sb = wpool.tile([128, CC, INNER], BF16, tag="wv")
            wo_sb = wpool.tile([128, IC, CDIM], BF16, tag="wo")
            nc.gpsimd.dma_start(wq_sb, wq.rearrange("(cc p) i -> p cc i", p=128))
            nc.gpsimd.dma_start(wk_sb, wk.rearrange("(cc p) i -> p cc i", p=128))
            nc.gpsimd.dma_start(wv_sb, wv.rearrange("(cc p) i -> p cc i", p=128))
            nc.gpsimd.dma_start(wo_sb, wo.rearrange("(ic p) o -> p ic o", p=128))

            bo_sb = wpool.tile([128, CDIM], F32, tag="bo")
            nc.sync.dma_start(
                bo_sb,
                bass.AP(tensor=bo[:].tensor, offset=0, ap=[[0, 128], [1, CDIM]]),
            )

            pixT = apool.tile([128, CC, NSH], BF16, tag="pixT")
            patT = apool.tile([128, CC, M], BF16, tag="patT")
            pix_r = pixelT.rearrange("(cc p) n -> p cc n", p=128)
            pat_r = patchT.rearrange("(cc p) m -> p cc m", p=128)
            for cc in range(CC):
                nc.sync.dma_start(pixT[:, cc, :], pix_r[:, cc, :])
            for mj in range(MJ):
                for cc in range(CC):
                    sl = slice(mj * 512, (mj + 1) * 512)
                    nc.sync.dma_start(patT[:, cc, sl], pat_r[:, cc, sl])

            # warm the exp table early so the first real exp isn't gated on it
            warm = rpool.tile([1, 16], BF16, tag="warm")
            nc.scalar.activation(
                warm, bo_sb[0:1, 0:16], mybir.ActivationFunctionType.Exp
            )

            qT = qkvpool.tile([128, IC, NSH], BF16, tag="qT")
            kT = qkvpool.tile([128, IC, M], BF16, tag="kT")
            outT = qkvpool.tile([128, IC, NSH], BF16, tag="outT")
            # v_sb: [m-chunk 128, head, 128] = [V_h | 1 | zeros] — col 64 gives
            # the softmax denominator via the matmul, cols 65..127 pad M to 128.
            v_sb = [
                vpool.tile([128, H, 128], BF16, tag="v", name=f"v{mi}")
                for mi in range(MT)
            ]
            for mi in range(MT):
                nc.vector.memset(v_sb[mi][:, :, D : 2 * D], 0.0)
                nc.vector.memset(v_sb[mi][:, :, D : D + 1], 1.0)

            with (
                tc.tile_pool(name="mmps", bufs=2, space="PSUM") as mmps,
                tc.tile_pool(name="sps", bufs=2, space="PSUM") as sps,
                tc.tile_pool(name="ops", bufs=1, space="PSUM") as ops,
            ):
                # ---- projection fillers ----------------------------------
                def fQ(ic, nj):
                    def run():
                        ps = mmps.tile([128, 512], F32, tag="mm", name=f"pq{ic}{nj}")
                        for cc in range(CC):
                            nc.tensor.matmul(
                                ps,
                                wq_sb[:, cc, ic * 128 : (ic + 1) * 128],
                                pixT[:, cc, nj * 512 : (nj + 1) * 512],
                                start=(cc == 0),
                                stop=(cc == CC - 1),
                            )
                        nc.vector.tensor_copy(
                            qT[:, ic, nj * 512 : (nj + 1) * 512], ps
                        )
                    return run

                def fK(ic, mj):
                    def run():
                        ps = mmps.tile([128, 512], F32, tag="mm", name=f"pk{ic}{mj}")
                        for cc in range(CC):
                            nc.tensor.matmul(
                                ps,
                                wk_sb[:, cc, ic * 128 : (ic + 1) * 128],
                                patT[:, cc, mj * 512 : (mj + 1) * 512],
                                start=(cc == 0),
                                stop=(cc == CC - 1),
                            )
                        nc.vector.tensor_copy(
                            kT[:, ic, mj * 512 : (mj + 1) * 512], ps
                        )
                    return run

                def fV(mi):
                    def run():
                        ps = mmps.tile([128, 512], F32, tag="mm", name=f"pv{mi}")
                        for cc in range(CC):
                            nc.tensor.matmul(
                                ps,
                                patT[:, cc, mi * 128 : (mi + 1) * 128],
                                wv_sb[:, cc, :],
                                start=(cc == 0),
                                stop=(cc == CC - 1),
                            )
                        nc.vector.tensor_copy(
                            v_sb[mi][:, :, 0:D],
                            ps.rearrange("p (h d) -> p h d", h=H),
                        )
                    return run

                # prefix: just enough for (nj0, pair0) to start
                fQ(0, 0)()
                for mj in range(MJ):
                    fK(0, mj)()
                fV(0)()
                fV(1)()

                fillers = {
                    (0, 0): [fV(mi) for mi in range(2, MT)] + [fQ(1, 0), fK(1, 0)],
                    (0, 1): [fK(1, 1), fK(1, 2), fK(1, 3), fQ(2, 0), fK(2, 0),
                             fK(2, 1), fK(2, 2), fK(2, 3)],
                    (0, 2): [fQ(3, 0), fK(3, 0), fK(3, 1), fK(3, 2), fK(3, 3),
                             fQ(0, 1)],
                    (0, 3): [fQ(1, 1), fQ(2, 1), fQ(3, 1)],
                }

                # ---- attention (per nj, head-pair) -----------------------
                def attention_block(nj, p):
                    ic = p
                    nsl = slice(nj * 512, (nj + 1) * 512)
                    fill = fillers.get((nj, p), [])
                    o = ops.tile([128, 1024], F32, tag="o", name=f"o{nj}{p}")
                    at_tiles = {}

                    def emit_S(k):
                        s = sps.tile(
                            [128, 1024], F32, tag="s", name=f"s{nj}{p}{k}"
                        )
                        ksl = slice(k * 128, (k + 1) * 128)
                        nc.tensor.matmul(
                            s[:, 0:512],
                            kT[0:D, ic, ksl],
                            qT[0:D, ic, nsl],
                            start=True, stop=True,
                            tile_position=(0, 0),
                        )
                        nc.tensor.matmul(
                            s[:, 512:1024],
                            kT[D : 2 * D, ic, ksl],
                            qT[D : 2 * D, ic, nsl],
                            start=True, stop=True,
                            tile_position=(64, 0),
                        )
                        at = atpool.tile(
                            [128, 1024], BF16, tag="at", name=f"at{nj}{p}{k}"
                        )
                        nc.scalar.activation(
                            at, s, mybir.ActivationFunctionType.Exp, scale=SCALE
                        )
                        at_tiles[k] = at

                    def emit_A(k):
                        at = at_tiles.pop(k)
                        nc.tensor.matmul(
                            o[:, 0:512],
                            v_sb[k][:, 2 * ic, :],
                            at[:, 0:512],
                            start=(k == 0),
                            stop=(k == MT - 1),
                        )
                        nc.tensor.matmul(
                            o[:, 512:1024],
                            v_sb[k][:, 2 * ic + 1, :],
                            at[:, 512:1024],
                            start=(k == 0),
                            stop=(k == MT - 1),
                        )

                    emit_S(0)
                    emit_S(1)
                    fi = 0
                    for k in range(MT):
                        if fi < len(fill):
                            fill[fi]()
                            fi += 1
                        if k + 2 < MT:
                            emit_S(k + 2)
                        emit_A(k)
                    while fi < len(fill):
                        fill[fi]()
                        fi += 1

                    # normalization: free o fast via SBUF copy, then divide
                    oraw = npool.tile([D + 1, 1024], F32, tag="oraw",
                                      name=f"or{nj}{p}")
                    nc.vector.tensor_copy(oraw, o[0 : D + 1, :])
                    r = npool.tile([1, 1024], F32, tag="r", name=f"r{nj}{p}")
                    nc.vector.reciprocal(r, oraw[D : D + 1, :])
                    r64 = npool.tile([D, 1024], F32, tag="r64",
                                     name=f"r64{nj}{p}")
                    r_ap = r[0:1, :]
                    nc.sync.dma_start(
                        r64,
                        bass.AP(
                            tensor=r_ap.tensor,
                            offset=r_ap.offset,
                            ap=[[1024, 1], [0, D], [1, 1024]],
                        ),
                    )
                    nc.vector.tensor_mul(
                        outT[0:D, ic, nsl], oraw[0:D, 0:512], r64[:, 0:512]
                    )
                    nc.vector.tensor_mul(
                        outT[D : 2 * D, ic, nsl],
                        oraw[0:D, 512:1024],
                        r64[:, 512:1024],
                    )

                def out_proj(nj):
                    for ni in range(nj * 4, nj * 4 + 4):
                        ps = mmps.tile([128, CDIM], F32, tag="mm",
                                       name=f"po{ni}")
                        for ic2 in range(IC):
                            nc.tensor.matmul(
                                ps,
                                outT[:, ic2, ni * 128 : (ni + 1) * 128],
                                wo_sb[:, ic2, :],
                                start=(ic2 == 0),
                                stop=(ic2 == IC - 1),
                            )
                        st = stpool.tile([128, CDIM], F32, tag="st",
                                         name=f"st{ni}")
                        nc.vector.tensor_add(st, ps, bo_sb)
                        nc.sync.dma_start(out[ni * 128 : (ni + 1) * 128, :], st)

                for nj in range(NJ):
                    for p in range(4):
                        attention_block(nj, p)
                    out_proj(nj)

    nc.finalize()
    return nc


def make_in_maps(pixel_embed, patch_embed, Wq, Wk, Wv, Wo, bo):
    bf = ml_dtypes.bfloat16
    pixel_embed = np.asarray(pixel_embed, dtype=np.float32)
    patch_embed = np.asarray(patch_embed, dtype=np.float32)
    wq = np.asarray(Wq, dtype=np.float32).astype(bf)
    wk = np.asarray(Wk, dtype=np.float32).astype(bf)
    wv = np.asarray(Wv, dtype=np.float32).astype(bf)
    wo = np.asarray(Wo, dtype=np.float32).astype(bf)
    bo = np.asarray(bo, dtype=np.float32)

    in_maps = []
    for core in range(N_CORES):
        bi, half = divmod(core, 2)
        px = pixel_embed[bi, half * NSH : (half + 1) * NSH, :]  # [NSH, CDIM]
        pa = patch_embed[bi]  # [M, CDIM]
        in_maps.append(
            {
                "pixelT": px.T.astype(bf),
                "patchT": pa.T.astype(bf),
                "wq": wq,
                "wk": wk,
                "wv": wv,
                "wo": wo,
                "bo": bo,
            }
        )
    return in_maps


def gather_out(results):
    out = np.empty((B, N, CDIM), np.float32)
    for core in range(N_CORES):
        bi, half = divmod(core, 2)
        out[bi, half * NSH : (half + 1) * NSH, :] = results[core]["out"]
    return out


_NC_CACHE = {}


def kernel(pixel_embed, patch_embed, Wq, Wk, Wv, Wo, bo, **kw):
    if "nc" not in _NC_CACHE:
        _NC_CACHE["nc"] = build_nc()
    nc = _NC_CACHE["nc"]
    in_maps = make_in_maps(pixel_embed, patch_embed, Wq, Wk, Wv, Wo, bo)
    res = run_bass_kernel_spmd(nc, in_maps, core_ids=list(range(N_CORES)), **kw)
    out = gather_out(res.results)
    if kw.get("trace"):
        return out, res
    return out


# revision 10
# speedup vs baseline: 1.2963x; 1.2963x over previous
"""Cross-attention kernel for Trainium2, sharded over 8 NeuronCores.

Problem (hardcoded): b=4, n=m=2048, query_dim=context_dim=512,
heads=8, dim_head=64 (inner=512), f32 I/O.

Sharding: data-parallel over (batch, query-half): core c -> batch c//2,
query rows [(c%2)*1024, (c%2+1)*1024). Each core holds the full K/V
context for its batch, so there are no collectives and output shards
tile the full output exactly.

v2 design (ACT-bound pipeline):
  - The softmax exp is the hard floor: 8 heads * 1024n * 2048m = 16.7M
    elements through ScalarE at 1 elem/lane/cycle @1.2GHz ~= 133us.
    Everything else is organized to hide under a continuous ACT stream.
  - Scores per head-pair via two row-tiled (K=64) matmuls running
    CONCURRENTLY in the PE array (tile_position (0,0)/(64,0)): head 2ic
    lives in partitions 0:64 of qT/kT, head 2ic+1 in 64:128. Halves
    score PE time vs the zero-padded full-K form.
  - nj-outer loop (two 512-query passes); per (nj, pair, mi) one exp
    instruction covers both heads [128, 1024].
  - Software pipelining: scores run 2 m-chunks ahead of attn@V so the
    PE never waits on exp; Q/K/V projections are emitted as fillers
    inside the first pair's attention stream.
  - attn@V keeps the ones-column trick: v tiles [128, h, 128] =
    [V_h | 1 | 0pad]; psum row 64 = softmax denominator.
  - PSUM budget (8 banks): scores 2x[128,1024] (4) + o [128,1024] (2)
    + proj/outproj [128,512] x2 (2). o is freed fast via a DVE copy to
    SBUF; normalization (recip + DMA broadcast + mul) runs from SBUF.
"""

import numpy as np
import ml_dtypes

import concourse.bass as bass
import concourse.mybir as mybir
import concourse.tile as tile
from concourse import bacc
from concourse.bass_utils import run_bass_kernel_spmd

BF16 = mybir.dt.bfloat16
F32 = mybir.dt.float32

B, N, M = 4, 2048, 2048
CDIM, INNER = 512, 512
H, D = 8, 64
NSH = N // 2  # query rows per core
N_CORES = 8
SCALE = D ** -0.5

CC = CDIM // 128   # contraction chunks for projections (4)
IC = INNER // 128  # inner-dim chunks (4)
MT = M // 128      # m tiles (16)
NJ = NSH // 512    # n chunks of 512 (2)
NT = NSH // 128    # n tiles (8)
MJ = M // 512      # m chunks of 512 (4)


def build_nc() -> bass.Bass:
    nc = bacc.Bacc(None)

    # all inputs are host-transposed into partition-major, per-chunk
    # contiguous layouts so every load is a linear DMA (big descriptors).
    pixelT = nc.dram_tensor("pixelT", [128, NJ, CC, 512], BF16, kind="ExternalInput")
    patchT = nc.dram_tensor("patchT", [128, MJ, CC, 512], BF16, kind="ExternalInput")
    wq = nc.dram_tensor("wq", [128, CC, INNER], BF16, kind="ExternalInput")
    wk = nc.dram_tensor("wk", [128, CC, INNER], BF16, kind="ExternalInput")
    wv = nc.dram_tensor("wv", [128, CC, INNER], BF16, kind="ExternalInput")
    wo = nc.dram_tensor("wo", [128, IC, CDIM], BF16, kind="ExternalInput")
    bo = nc.dram_tensor("bo", [CDIM], F32, kind="ExternalInput")
    out = nc.dram_tensor("out", [NSH, CDIM], F32, kind="ExternalOutput")

    with tile.TileContext(nc) as tc:
        with (
            tc.tile_pool(name="weights", bufs=1) as wpool,
            tc.tile_pool(name="acts", bufs=1) as apool,
            tc.tile_pool(name="qkv", bufs=1) as qkvpool,
            tc.tile_pool(name="vsb", bufs=1) as vpool,
            tc.tile_pool(name="attn", bufs=4) as atpool,
            tc.tile_pool(name="norm", bufs=2) as npool,
            tc.tile_pool(name="small", bufs=2) as rpool,
            tc.tile_pool(name="stage", bufs=2) as stpool,
        ):
            # ---- load weights + activations -------------------------------
            wq_sb = wpool.tile([128, CC, INNER], BF16, tag="wq")
            wk_sb = wpool.tile([128, CC, INNER], BF16, tag="wk")
            wv_sb = wpool.tile([128, CC, INNER], BF16, tag="wv")
            wo_sb = wpool.tile([128, IC, CDIM], BF16, tag="wo")
            nc.gpsimd.dma_start(wq_sb, wq[:, :, :])
            nc.gpsimd.dma_start(wk_sb, wk[:, :, :])
            nc.gpsimd.dma_start(wv_sb, wv[:, :, :])

            bo1 = wpool.tile([1, CDIM], F32, tag="bo1")
            nc.sync.dma_start(
                bo1,
                bass.AP(tensor=bo[:].tensor, offset=0, ap=[[0, 1], [1, CDIM]]),
            )
            bo_sb = wpool.tile([128, CDIM], F32, tag="bo")
            nc.gpsimd.partition_broadcast(bo_sb, bo1)

            pixT = apool.tile([128, NJ, CC, 512], BF16, tag="pixT")
            patT = apool.tile([128, MJ, CC, 512], BF16, tag="patT")
            nc.sync.dma_start(pixT[:, 0], pixelT[:, 0])
            nc.sync.dma_start(patT[:, 0], patchT[:, 0])
            nc.sync.dma_start(pixT[:, 1], pixelT[:, 1])
            for mj in range(1, MJ):
                nc.scalar.dma_start(patT[:, mj], patchT[:, mj])

            nc.gpsimd.dma_start(wo_sb, wo[:, :, :])

            # warm the exp table early so the first real exp isn't gated on it
            warm = rpool.tile([1, 16], BF16, tag="warm")
            nc.scalar.activation(
                warm, bo1[0:1, 0:16], mybir.ActivationFunctionType.Exp
            )

            qT = qkvpool.tile([128, IC, NSH], BF16, tag="qT")
            kT = qkvpool.tile([128, IC, M], BF16, tag="kT")
            outT = qkvpool.tile([128, IC, NSH], BF16, tag="outT")
            # v_sb: [m-chunk 128, head, 128] = [V_h | 1 | zeros] — col 64 gives
            # the softmax denominator via the matmul, cols 65..127 pad M to 128.
            v_sb = vpool.tile([128, MT, H, 128], BF16, tag="v")
            nc.vector.memset(v_sb[:, :, :, D : 2 * D], 0.0)
            nc.vector.memset(v_sb[:, :, :, D : D + 1], 1.0)

            with (
                tc.tile_pool(name="mmps", bufs=2, space="PSUM") as mmps,
                tc.tile_pool(name="sps", bufs=2, space="PSUM") as sps,
                tc.tile_pool(name="ops", bufs=1, space="PSUM") as ops,
            ):
                # ---- projection fillers ----------------------------------
                def fQ(ic, nj):
                    def run():
                        ps = mmps.tile([128, 512], F32, tag="mm", name=f"pq{ic}{nj}")
                        for cc in range(CC):
                            nc.tensor.matmul(
                                ps,
                                wq_sb[:, cc, ic * 128 : (ic + 1) * 128],
                                pixT[:, nj, cc, :],
                                start=(cc == 0),
                                stop=(cc == CC - 1),
                            )
                        nc.vector.tensor_copy(
                            qT[:, ic, nj * 512 : (nj + 1) * 512], ps
                        )
                    return run

                def fK(ic, mj):
                    def run():
                        ps = mmps.tile([128, 512], F32, tag="mm", name=f"pk{ic}{mj}")
                        for cc in range(CC):
                            nc.tensor.matmul(
                                ps,
                                wk_sb[:, cc, ic * 128 : (ic + 1) * 128],
                                patT[:, mj, cc, :],
                                start=(cc == 0),
                                stop=(cc == CC - 1),
                            )
                        nc.vector.tensor_copy(
                            kT[:, ic, mj * 512 : (mj + 1) * 512], ps
                        )
                    return run

                def fV(mi):
                    def run():
                        ps = mmps.tile([128, 512], F32, tag="mm", name=f"pv{mi}")
                        for cc in range(CC):
                            nc.tensor.matmul(
                                ps,
                                patT[:, mi // 4, cc,
                                     (mi % 4) * 128 : (mi % 4 + 1) * 128],
                                wv_sb[:, cc, :],
                                start=(cc == 0),
                                stop=(cc == CC - 1),
                            )
                        nc.vector.tensor_copy(
                            v_sb[:, mi, :, 0:D],
                            ps.rearrange("p (h d) -> p h d", h=H),
                        )
                    return run

                # prefix: just enough for (nj0, pair0, mi0..3) to start
                fQ(0, 0)()
                fK(0, 0)()
                fV(0)()
                fV(1)()

                fillers = {
                    (0, 0): [[fK(0, 1), fV(2)], [fK(0, 2), fV(3)],
                             [fK(0, 3), fV(4)], [fV(5), fV(6)]]
                            + [[fV(mi)] for mi in range(7, MT)]
                            + [[fQ(1, 0)], [fK(1, 0)], []],
                    (0, 1): [[fK(1, 1)], [fK(1, 2)], [fK(1, 3)], [fQ(2, 0)],
                             [fK(2, 0)], [fK(2, 1)], [fK(2, 2)], [fK(2, 3)]],
                    (0, 2): [[fQ(3, 0)], [fK(3, 0)], [fK(3, 1)], [fK(3, 2)],
                             [fK(3, 3)], [fQ(0, 1)]],
                    (0, 3): [[fQ(1, 1)], [fQ(2, 1)], [fQ(3, 1)]],
                }

                # ---- attention (per nj, head-pair) -----------------------
                def attention_block(nj, p):
                    ic = p
                    nsl = slice(nj * 512, (nj + 1) * 512)
                    fill = fillers.get((nj, p), [])
                    o = ops.tile([128, 1024], F32, tag="o", name=f"o{nj}{p}")
                    at_tiles = {}

                    def emit_S(k):
                        s = sps.tile(
                            [128, 1024], F32, tag="s", name=f"s{nj}{p}{k}"
                        )
                        ksl = slice(k * 128, (k + 1) * 128)
                        nc.tensor.matmul(
                            s[:, 0:512],
                            kT[0:D, ic, ksl],
                            qT[0:D, ic, nsl],
                            start=True, stop=True,
                            tile_position=(0, 0),
                        )
                        nc.tensor.matmul(
                            s[:, 512:1024],
                            kT[D : 2 * D, ic, ksl],
                            qT[D : 2 * D, ic, nsl],
                            start=True, stop=True,
                            tile_position=(64, 0),
                        )
                        at = atpool.tile(
                            [128, 1024], BF16, tag="at", name=f"at{nj}{p}{k}"
                        )
                        nc.scalar.activation(
                            at, s, mybir.ActivationFunctionType.Exp, scale=SCALE
                        )
                        at_tiles[k] = at

                    def emit_A(k):
                        at = at_tiles.pop(k)
                        nc.tensor.matmul(
                            o[:, 0:512],
                            v_sb[:, k, 2 * ic, :],
                            at[:, 0:512],
                            start=(k == 0),
                            stop=(k == MT - 1),
                        )
                        nc.tensor.matmul(
                            o[:, 512:1024],
                            v_sb[:, k, 2 * ic + 1, :],
                            at[:, 512:1024],
                            start=(k == 0),
                            stop=(k == MT - 1),
                        )

                    emit_S(0)
                    emit_S(1)
                    for k in range(MT):
                        if k < len(fill):
                            for f in fill[k]:
                                f()
                        if k + 2 < MT:
                            emit_S(k + 2)
                        emit_A(k)

                    # normalization: copy o out fast (frees psum), recip of
                    # row 64, broadcast via Pool engine, divide into outT.
                    oraw = npool.tile([D + 1, 1024], F32, tag="oraw",
                                      name=f"or{nj}{p}")
                    nc.vector.tensor_copy(oraw, o[0 : D + 1, :])
                    r = npool.tile([1, 1024], F32, tag="r", name=f"r{nj}{p}")
                    nc.vector.reciprocal(r, oraw[D : D + 1, :])
                    r64 = npool.tile([D, 1024], F32, tag="r64",
                                     name=f"r64{nj}{p}")
                    nc.gpsimd.partition_broadcast(r64, r[0:1, :])
                    nc.gpsimd.tensor_mul(
                        outT[0:D, ic, nsl], oraw[0:D, 0:512], r64[:, 0:512]
                    )
                    nc.gpsimd.tensor_mul(
                        outT[D : 2 * D, ic, nsl],
                        oraw[0:D, 512:1024],
                        r64[:, 512:1024],
                    )

                def fPO(ni):
                    def run():
                        ps = mmps.tile([128, CDIM], F32, tag="mm",
                                       name=f"po{ni}")
                        ic_order = [3, 0, 1, 2]
                        for j, ic2 in enumerate(ic_order):
                            nc.tensor.matmul(
                                ps,
                                outT[:, ic2, ni * 128 : (ni + 1) * 128],
                                wo_sb[:, ic2, :],
                                start=(j == 0),
                                stop=(j == IC - 1),
                            )
                        st = stpool.tile([128, CDIM], F32, tag="st",
                                         name=f"st{ni}")
                        nc.vector.tensor_add(st, ps, bo_sb)
                        nc.sync.dma_start(out[ni * 128 : (ni + 1) * 128, :], st)
                    return run

                fillers[(1, 0)] = [[], [], [], [], [fPO(0)], [], [fPO(1)],
                                   [], [fPO(2)], [], [fPO(3)]]

                for nj in range(NJ):
                    for p in range(4):
                        attention_block(nj, p)
                for ni in range(4, 8):
                    fPO(ni)()

    nc.finalize()
    return nc


def make_in_maps(pixel_embed, patch_embed, Wq, Wk, Wv, Wo, bo):
    bf = ml_dtypes.bfloat16
    pixel_embed = np.asarray(pixel_embed, dtype=np.float32)
    patch_embed = np.asarray(patch_embed, dtype=np.float32)
    wq = np.asarray(Wq, dtype=np.float32).astype(bf)
    wk = np.asarray(Wk, dtype=np.float32).astype(bf)
    wv = np.asarray(Wv, dtype=np.float32).astype(bf)
    wo = np.asarray(Wo, dtype=np.float32).astype(bf)
    bo = np.asarray(bo, dtype=np.float32)

    # host-side relayouts so every device DMA is contiguous (see build_nc)
    def chunkT(a, j):  # [rows, j*512] -> [128, j, rows//128, 512]
        r = a.shape[0]
        return np.ascontiguousarray(
            a.reshape(r // 128, 128, j, 512).transpose(1, 2, 0, 3)
        )

    def wchunk(w):  # [512, out] -> [128, 4, out]
        return np.ascontiguousarray(
            w.reshape(4, 128, w.shape[1]).transpose(1, 0, 2)
        )

    wq, wk, wv, wo = wchunk(wq), wchunk(wk), wchunk(wv), wchunk(wo)
    in_maps = []
    for core in range(N_CORES):
        bi, half = divmod(core, 2)
        px = pixel_embed[bi, half * NSH : (half + 1) * NSH, :]  # [NSH, CDIM]
        pa = patch_embed[bi]  # [M, CDIM]
        in_maps.append(
            {
                "pixelT": chunkT(px.T.astype(bf), NJ),
                "patchT": chunkT(pa.T.astype(bf), MJ),
                "wq": wq,
                "wk": wk,
                "wv": wv,
                "wo": wo,
                "bo": bo,
            }
        )
    return in_maps


def gather_out(results):
    out = np.empty((B, N, CDIM), np.float32)
    for core in range(N_CORES):
        bi, half = divmod(core, 2)
        out[bi, half * NSH : (half + 1) * NSH, :] = results[core]["out"]
    return out


_NC_CACHE = {}


def kernel(pixel_embed, patch_embed, Wq, Wk, Wv, Wo, bo, **kw):
    if "nc" not in _NC_CACHE:
        _NC_CACHE["nc"] = build_nc()
    nc = _NC_CACHE["nc"]
    in_maps = make_in_maps(pixel_embed, patch_embed, Wq, Wk, Wv, Wo, bo)
    res = run_bass_kernel_spmd(nc, in_maps, core_ids=list(range(N_CORES)), **kw)
    out = gather_out(res.results)
    if kw.get("trace"):
        return out, res
    return out
